# revision 10
# baseline (speedup 1.0000x reference)
"""Fused transformer block (RMSNorm + qk-norm attention + MLP) for TRN2, 8 cores.

Sharding: 8 cores = (4 batches) x (2 query-halves). Each core gets its batch's
full sequence with rows rotated so its query half is rows 0..1023 (attention is
permutation-invariant over keys, so K/V row order doesn't matter). No
collectives needed; each core produces a disjoint [1024, 768] output slice.

Layout strategy per core:
  - x_hat = rmsnorm(lat) computed in natural [s, d] layout, cast bf16, round-
    tripped through DRAM with DMA-transpose to get x_hat^T [d, s] for matmuls.
  - Q/K projections in natural layout (lhsT = x_hat^T tile), qk-rmsnorm applied
    in natural layout, then DMA-transposed to Q^T/K^T [hd, s] per head pair.
  - logits^T[k, q] = K^T_h.T @ Q^T_h per (head, k-tile); exp on ScalarE
    (no max subtraction: |logit| <= 8 since q,k are unit-RMS), P in [k, q]
    layout directly feeds attn@V with V in natural [k, hd] layout augmented
    with a ones column -> softmax denominator lands in PSUM row 64 for free.
  - out-proj / MLP2 use the activation tile as the stationary operand so the
    result comes out in natural [q, d] layout for residuals.
"""

import numpy as np
from contextlib import ExitStack

import concourse.bass as bass
import concourse.tile as tile
from concourse import bacc, mybir
from concourse.bass_utils import run_bass_kernel_spmd

F32 = mybir.dt.float32
BF16 = mybir.dt.bfloat16
AF = mybir.ActivationFunctionType
OP = mybir.AluOpType

B, S, D, H, HD, MLP = 4, 2048, 768, 12, 64, 3072
SQ = S // 2            # query rows per core
NT_S = S // 128        # 16 sequence tiles
NT_Q = SQ // 128       # 8 query tiles
NT_D = D // 128        # 6 model-dim tiles
NT_M = MLP // 128      # 24 mlp-dim tiles
EPS = 1e-6
VW = HD + 1            # V width incl. ones column


def _chunks(n):
    # split free dim n into <=512 matmul chunks
    out, ofs = [], 0
    while ofs < n:
        c = min(512, n - ofs)
        out.append((ofs, c))
        ofs += c
    return out


def build_nc(sim_compat=False):
    nc = bacc.Bacc("TRN2", target_bir_lowering=False, debug=False, num_devices=8)

    lat = nc.dram_tensor("lat", [S, D], F32, kind="ExternalInput").ap()
    wq = nc.dram_tensor("wq", [D, D], BF16, kind="ExternalInput").ap()
    wk = nc.dram_tensor("wk", [D, D], BF16, kind="ExternalInput").ap()
    wv = nc.dram_tensor("wv", [D, D], BF16, kind="ExternalInput").ap()
    wo = nc.dram_tensor("wo", [D, D], BF16, kind="ExternalInput").ap()
    wi = nc.dram_tensor("wi", [D, MLP], BF16, kind="ExternalInput").ap()
    wom = nc.dram_tensor("wom", [MLP, D], BF16, kind="ExternalInput").ap()
    kqsc = nc.dram_tensor("kqsc", [128, 1], F32, kind="ExternalInput").ap()
    out = nc.dram_tensor("out", [SQ, D], F32, kind="ExternalOutput").ap()

    with tile.TileContext(nc) as tc, ExitStack() as top:
        def ptile(pool, shape, dtype, name):
            return pool.tile(shape, dtype, name=name, tag=name)

        p_const = top.enter_context(tc.tile_pool(name="p_const", bufs=1))
        p_x2 = top.enter_context(tc.tile_pool(name="p_x2", bufs=1))
        p_oT = tc.alloc_tile_pool(name="p_oT", bufs=1)
        p_att = tc.alloc_tile_pool(name="p_att", bufs=1)

        # ---- persistent tiles ----
        Vaug = ptile(p_att, [128, NT_S * H * VW], BF16, name="Vaug")
        oT = ptile(p_oT, [128, NT_D * SQ], BF16, name="oT")
        kqsc_t = ptile(p_const, [128, 1], F32, name="kqsc_t")
        ones1 = ptile(p_const, [1, 64], F32, name="ones1")
        KT = [ptile(p_att, [128, S], BF16, name=f"KT{d}") for d in range(NT_D)]
        QT = [ptile(p_att, [128, SQ], BF16, name=f"QT{d}") for d in range(NT_D)]
        x2 = [ptile(p_x2, [128, D], F32, name=f"x2_{q}") for q in range(NT_Q)]
        x2T = [ptile(p_x2, [128, SQ], BF16, name=f"x2T{d}") for d in range(NT_D)]

        nc.sync.dma_start(kqsc_t[:], kqsc[:])
        eps_t = ptile(p_const, [128, 1], F32, name="eps_t")
        nc.vector.memset(eps_t[:], EPS)
        nc.vector.memset(ones1[:], 1.0)
        # ones columns of Vaug (written once; V copies only touch the 64-wide blocks)
        vview = Vaug[:].rearrange("p (s h k) -> p s h k", s=NT_S, h=H)
        nc.vector.memset(vview[:, :, :, HD:VW], 1.0)

        dram = top.enter_context(tc.tile_pool(name="dram", bufs=1, space="DRAM"))
        xh_d = dram.tile([S, D], BF16, name="xh_d")
        kh_d = dram.tile([S, D], BF16, name="kh_d")
        qh_d = dram.tile([SQ, D], BF16, name="qh_d")
        x2h_d = dram.tile([SQ, D], BF16, name="x2h_d")

        # =============== Phase A: ln1 + x_hat^T ===============
        with ExitStack() as ctx:
            io = ctx.enter_context(tc.tile_pool(name="a_io", bufs=3))
            st_p = ctx.enter_context(tc.tile_pool(name="a_stats", bufs=4))
            scr = ctx.enter_context(tc.tile_pool(name="a_scr", bufs=3))
            for t in range(NT_S):
                lt = io.tile([128, D], F32, name="lt")
                nc.sync.dma_start(lt[:], lat[t * 128:(t + 1) * 128, :])
                sq = scr.tile([128, D], F32, name="sq")
                ssq = st_p.tile([128, 1], F32, name="ssq")
                nc.scalar.activation(sq[:], lt[:], AF.Square, accum_out=ssq[:])
                srt = st_p.tile([128, 1], F32, name="srt")
                nc.scalar.activation(srt[:], ssq[:], AF.Sqrt, bias=eps_t[:], scale=1.0 / D)
                rs = st_p.tile([128, 1], F32, name="rs")
                nc.vector.reciprocal(rs[:], srt[:])
                xh = scr.tile([128, D], BF16, name="xh")
                nc.vector.tensor_scalar_mul(xh[:], lt[:], rs[:])
                nc.sync.dma_start(xh_d[t * 128:(t + 1) * 128, :], xh[:])

        p_xT = tc.alloc_tile_pool(name="p_xT", bufs=1)
        xT = [ptile(p_xT, [128, S], BF16, name=f"xT{d}") for d in range(NT_D)]
        for d in range(NT_D):
            nc.sync.dma_start_transpose(xT[d][:], xh_d[:, d * 128:(d + 1) * 128])

        # =============== Phase B: Q/K/V projections + qk-norm ===============
        with ExitStack() as ctx:
            wp = ctx.enter_context(tc.tile_pool(name="b_w", bufs=1))
            wq_sb = [wp.tile([128, D], BF16, name=f"wq_sb{d}") for d in range(NT_D)]
            wk_sb = [wp.tile([128, D], BF16, name=f"wk_sb{d}") for d in range(NT_D)]
            wv_sb = [wp.tile([128, D], BF16, name=f"wv_sb{d}") for d in range(NT_D)]
            for d in range(NT_D):
                nc.sync.dma_start(wq_sb[d][:], wq[d * 128:(d + 1) * 128, :])
                nc.sync.dma_start(wk_sb[d][:], wk[d * 128:(d + 1) * 128, :])
                nc.sync.dma_start(wv_sb[d][:], wv[d * 128:(d + 1) * 128, :])

            ps = ctx.enter_context(tc.tile_pool(name="b_ps", bufs=3, space="PSUM"))
            scr = ctx.enter_context(tc.tile_pool(name="b_scr", bufs=3))
            st_p = ctx.enter_context(tc.tile_pool(name="b_stats", bufs=6))
            natp = ctx.enter_context(tc.tile_pool(name="b_nat", bufs=3))

            def proj(t, w_sb, n_tiles):
                p = ps.tile([128, D], F32, name="p_proj")
                for d in range(NT_D):
                    for ofs, n in _chunks(D):
                        nc.tensor.matmul(
                            p[:, ofs:ofs + n],
                            xT[d][:, t * 128:(t + 1) * 128],
                            w_sb[d][:, ofs:ofs + n],
                            start=(d == 0), stop=(d == NT_D - 1))
                return p

            def qknorm(p, dst_dram, t):
                sq = scr.tile([128, D], F32, name="sq_b")
                nc.scalar.activation(sq[:], p[:], AF.Square)
                ss = st_p.tile([128, H], F32, name="ss_b")
                nc.vector.tensor_reduce(
                    ss[:], sq[:].rearrange("p (h k) -> p h k", h=H),
                    axis=mybir.AxisListType.X, op=OP.add)
                srt = st_p.tile([128, H], F32, name="srt_b")
                nc.scalar.activation(srt[:], ss[:], AF.Sqrt, bias=eps_t[:], scale=1.0 / HD)
                rs = st_p.tile([128, H], F32, name="rs_b")
                nc.vector.reciprocal(rs[:], srt[:])
                nat = natp.tile([128, D], BF16, name="nat_b")
                rs_view = rs[:].rearrange("p (h o) -> p h o", o=1).broadcast_to([128, H, HD])
                nc.vector.tensor_tensor(
                    out=nat[:].rearrange("p (h k) -> p h k", h=H),
                    in0=p[:].rearrange("p (h k) -> p h k", h=H),
                    in1=rs_view, op=OP.mult)
                nc.sync.dma_start(dst_dram[t * 128:(t + 1) * 128, :], nat[:])

            for t in range(NT_S):
                pk = proj(t, wk_sb, NT_S)
                qknorm(pk, kh_d, t)
                pv = proj(t, wv_sb, NT_S)
                nc.vector.tensor_copy(
                    vview[:, t, :, 0:HD],
                    pv[:].rearrange("p (h k) -> p h k", h=H))
            for t in range(NT_Q):
                pq = proj(t, wq_sb, NT_Q)
                qknorm(pq, qh_d, t)

        p_xT.release()
        for d in range(NT_D):
            nc.sync.dma_start_transpose(KT[d][:], kh_d[:, d * 128:(d + 1) * 128])
            # fold q_norm_scale * k_norm_scale / sqrt(HD) into K^T (per-partition)
            nc.vector.tensor_scalar_mul(KT[d][:], KT[d][:], kqsc_t[:])
            nc.sync.dma_start_transpose(QT[d][:], qh_d[:, d * 128:(d + 1) * 128])

        # =============== Phase C: attention ===============
        with ExitStack() as ctx:
            psL = ctx.enter_context(tc.tile_pool(name="c_psL", bufs=2, space="PSUM"))
            psO = ctx.enter_context(tc.tile_pool(name="c_psO", bufs=1, space="PSUM"))
            psB = ctx.enter_context(tc.tile_pool(name="c_psB", bufs=1, space="PSUM"))
            pp = ctx.enter_context(tc.tile_pool(name="c_p", bufs=3))
            rp = ctx.enter_context(tc.tile_pool(name="c_r", bufs=2))

            for h in range(H):
                dt, base = h // 2, (h % 2) * 64
                o_ps = psO.tile([VW, SQ], F32, name="o_ps")
                for t in range(NT_S):
                    l_ps = psL.tile([128, SQ], F32, name="l_ps")
                    for ofs, n in _chunks(SQ):
                        nc.tensor.matmul(
                            l_ps[:, ofs:ofs + n],
                            KT[dt][base:base + 64, t * 128:(t + 1) * 128],
                            QT[dt][base:base + 64, ofs:ofs + n],
                            start=True, stop=True)
                    p_t = pp.tile([128, SQ], BF16, name="p_t")
                    nc.scalar.activation(p_t[:], l_ps[:], AF.Exp)
                    vofs = t * H * VW + h * VW
                    for ofs, n in _chunks(SQ):
                        nc.tensor.matmul(
                            o_ps[:, ofs:ofs + n],
                            Vaug[:, vofs:vofs + VW],
                            p_t[:, ofs:ofs + n],
                            start=(t == 0), stop=(t == NT_S - 1))
                rec = rp.tile([1, SQ], F32, name="rec")
                nc.vector.reciprocal(rec[:], o_ps[HD:VW, :])
                b_ps = psB.tile([64, SQ], F32, name="b_ps")
                for ofs, n in _chunks(SQ):
                    nc.tensor.matmul(b_ps[:, ofs:ofs + n], ones1[:],
                                     rec[:, ofs:ofs + n], start=True, stop=True)
                b_sb = rp.tile([64, SQ], F32, name="b_sb")
                nc.vector.tensor_copy(b_sb[:], b_ps[:])
                nc.vector.tensor_tensor(
                    out=oT[base:base + 64, dt * SQ:(dt + 1) * SQ],
                    in0=o_ps[0:HD, :], in1=b_sb[:], op=OP.mult)

        p_att.release()

        # =============== Phase D: out-proj + residual + ln2 ===============
        with ExitStack() as ctx:
            wp = ctx.enter_context(tc.tile_pool(name="d_w", bufs=1))
            wo_sb = [wp.tile([128, D], BF16, name=f"wo_sb{d}") for d in range(NT_D)]
            for d in range(NT_D):
                nc.sync.dma_start(wo_sb[d][:], wo[d * 128:(d + 1) * 128, :])
            ps = ctx.enter_context(tc.tile_pool(name="d_ps", bufs=2, space="PSUM"))
            io = ctx.enter_context(tc.tile_pool(name="d_io", bufs=3))
            scr = ctx.enter_context(tc.tile_pool(name="d_scr", bufs=3))
            st_p = ctx.enter_context(tc.tile_pool(name="d_stats", bufs=4))

            for q in range(NT_Q):
                p = ps.tile([128, D], F32, name="p_oproj")
                for d in range(NT_D):
                    for ofs, n in _chunks(D):
                        nc.tensor.matmul(
                            p[:, ofs:ofs + n],
                            oT[:, d * SQ + q * 128: d * SQ + (q + 1) * 128],
                            wo_sb[d][:, ofs:ofs + n],
                            start=(d == 0), stop=(d == NT_D - 1))
                lt = io.tile([128, D], F32, name="lt_d")
                nc.sync.dma_start(lt[:], lat[q * 128:(q + 1) * 128, :])
                nc.vector.tensor_tensor(out=x2[q][:], in0=p[:], in1=lt[:], op=OP.add)
                sq = scr.tile([128, D], F32, name="sq_d")
                ssq = st_p.tile([128, 1], F32, name="ssq_d")
                nc.scalar.activation(sq[:], x2[q][:], AF.Square, accum_out=ssq[:])
                srt = st_p.tile([128, 1], F32, name="srt_d")
                nc.scalar.activation(srt[:], ssq[:], AF.Sqrt, bias=eps_t[:], scale=1.0 / D)
                rs = st_p.tile([128, 1], F32, name="rs_d")
                nc.vector.reciprocal(rs[:], srt[:])
                xh2 = scr.tile([128, D], BF16, name="xh2")
                nc.vector.tensor_scalar_mul(xh2[:], x2[q][:], rs[:])
                nc.sync.dma_start(x2h_d[q * 128:(q + 1) * 128, :], xh2[:])

        p_oT.release()
        for d in range(NT_D):
            nc.sync.dma_start_transpose(x2T[d][:], x2h_d[:, d * 128:(d + 1) * 128])

        # =============== Phase E: MLP ===============
        p_hT = tc.alloc_tile_pool(name="p_hT", bufs=1)
        hT = ptile(p_hT, [128, NT_M * SQ], BF16, name="hT")
        with ExitStack() as ctx:
            wp = ctx.enter_context(tc.tile_pool(name="e_w", bufs=1))
            wi_sb = [wp.tile([128, MLP], BF16, name=f"wi_sb{d}") for d in range(NT_D)]
            for d in range(NT_D):
                nc.sync.dma_start(wi_sb[d][:], wi[d * 128:(d + 1) * 128, :])
            wom_sb = [wp.tile([128, D], BF16, name=f"wom_sb{m}") for m in range(NT_M)]
            for m in range(NT_M):
                nc.sync.dma_start(wom_sb[m][:], wom[m * 128:(m + 1) * 128, :])

            ps = ctx.enter_context(tc.tile_pool(name="e_ps", bufs=1, space="PSUM"))
            iop = ctx.enter_context(tc.tile_pool(name="e_io", bufs=3))

            for m in range(NT_M):
                p = ps.tile([128, SQ], F32, name="p_mlp1", bufs=2)
                for d in range(NT_D):
                    for ofs, n in _chunks(SQ):
                        nc.tensor.matmul(
                            p[:, ofs:ofs + n],
                            wi_sb[d][:, m * 128:(m + 1) * 128],
                            x2T[d][:, ofs:ofs + n],
                            start=(d == 0), stop=(d == NT_D - 1))
                if not sim_compat:
                    nc.scalar.activation(hT[:, m * SQ:(m + 1) * SQ], p[:],
                                         AF.Gelu_apprx_tanh)
                else:
                    # gelu(x) = 0.5*x*(1+tanh(sqrt(2/pi)*(x+0.044715*x^3)))
                    xsq = iop.tile([128, SQ], F32, name="g_xsq", bufs=1)
                    nc.vector.tensor_tensor(out=xsq[:], in0=p[:], in1=p[:], op=OP.mult)
                    w = iop.tile([128, SQ], F32, name="g_w", bufs=1)
                    nc.vector.tensor_scalar(w[:], xsq[:], 0.044715, 1.0,
                                            op0=OP.mult, op1=OP.add)
                    u = iop.tile([128, SQ], F32, name="g_u", bufs=1)
                    nc.vector.tensor_tensor(out=u[:], in0=w[:], in1=p[:], op=OP.mult)
                    th = iop.tile([128, SQ], F32, name="g_th", bufs=1)
                    nc.scalar.activation(th[:], u[:], AF.Tanh, scale=0.7978845608028654)
                    t2 = iop.tile([128, SQ], F32, name="g_t2", bufs=1)
                    nc.vector.scalar_tensor_tensor(t2[:], th[:], 1.0, p[:],
                                                   op0=OP.add, op1=OP.mult)
                    nc.vector.tensor_scalar_mul(hT[:, m * SQ:(m + 1) * SQ], t2[:], 0.5)

            for q in range(NT_Q):
                p = ps.tile([128, D], F32, name="p_mlp2", bufs=2)
                for m in range(NT_M):
                    for ofs, n in _chunks(D):
                        nc.tensor.matmul(
                            p[:, ofs:ofs + n],
                            hT[:, m * SQ + q * 128: m * SQ + (q + 1) * 128],
                            wom_sb[m][:, ofs:ofs + n],
                            start=(m == 0), stop=(m == NT_M - 1))
                ot = iop.tile([128, D], F32, name="ot_e")
                nc.vector.tensor_tensor(out=ot[:], in0=p[:], in1=x2[q][:], op=OP.add)
                nc.sync.dma_start(out[q * 128:(q + 1) * 128, :], ot[:])

        p_hT.release()

    nc.compile()
    return nc


def make_in_maps(latents, ln1_scale, wq, wk, wv, q_norm_scale, k_norm_scale,
                 wo_attn, ln2_scale, wi, wo_mlp):
    import ml_dtypes
    bf = ml_dtypes.bfloat16
    wq2 = (np.asarray(ln1_scale, np.float64)[:, None]
           * np.asarray(wq, np.float64).reshape(D, D)).astype(bf)
    wk2 = (np.asarray(ln1_scale, np.float64)[:, None]
           * np.asarray(wk, np.float64).reshape(D, D)).astype(bf)
    wv2 = (np.asarray(ln1_scale, np.float64)[:, None]
           * np.asarray(wv, np.float64).reshape(D, D)).astype(bf)
    wo2 = np.asarray(wo_attn, np.float32).reshape(D, D).astype(bf)
    wi2 = (np.asarray(ln2_scale, np.float64)[:, None]
           * np.asarray(wi, np.float64)).astype(bf)
    wom2 = np.asarray(wo_mlp, np.float32).astype(bf)
    kq = (np.tile(np.asarray(q_norm_scale, np.float64)
                  * np.asarray(k_norm_scale, np.float64), 2)
          / np.sqrt(HD)).astype(np.float32)[:, None]
    lat_np = np.asarray(latents, np.float32)
    in_maps = []
    for c in range(8):
        b, half = c // 2, c % 2
        lm = lat_np[b]
        lat_rot = np.concatenate([lm[half * SQ:(half + 1) * SQ],
                                  lm[(1 - half) * SQ:(2 - half) * SQ]], axis=0)
        in_maps.append(dict(lat=np.ascontiguousarray(lat_rot), wq=wq2, wk=wk2,
                            wv=wv2, wo=wo2, wi=wi2, wom=wom2, kqsc=kq))
    return in_maps


_NC_CACHE = None


def kernel(**inputs):
    global _NC_CACHE
    if _NC_CACHE is None:
        _NC_CACHE = build_nc()
    nc = _NC_CACHE
    in_maps = make_in_maps(**inputs)
    res = run_bass_kernel_spmd(nc, in_maps, list(range(8)))
    y = np.empty((B, S, D), np.float32)
    for c in range(8):
        b, half = c // 2, c % 2
        y[b, half * SQ:(half + 1) * SQ] = res.results[c]["out"]
    return y


if __name__ == "__main__":
    import reference
    inputs = {k: np.asarray(v) for k, v in reference.setup_inputs().items()}
    y = kernel(**inputs)
    exp = np.asarray(reference.reference(**reference.setup_inputs()))
    err = np.abs(y - exp).max() / np.abs(exp).max()
    print("Relative error:", err)
